# revision 32
# baseline (speedup 1.0000x reference)
"""Causal self-attention (B=2, S=2048, D=1024, H=16) on 8 trn2 NeuronCores.

Sharding: core c -> batch b = c//4, head-group hg = c%4 (4 heads/core).
Each core computes the qkv projection for its heads, causal attention, and a
partial output projection (rows hg*256:(hg+1)*256 of w_proj). The host sums
the 4 partials per batch and adds the bias terms.

v2 design (fp16 datapath, fp32 psum):
  x16   [128, 8, S]   fp16  x[b]^T d-chunked      (DMA'd in 4 column blocks)
  wqk16 [128, 8, 512] fp16  (q cols pre-scaled by 1/sqrt(64))
  wv16  [128, 8, 256] fp16  wp16 [128, 2, 1024] fp16
  qkT[m] [128, S] fp16  = (x @ wqk)^T m-block (q pairs m=0,1; k pairs 2,3)
  va2[t] [128, 2, 4, 65] fp16  v for s-tile pair t, 65th col = ones
  scores: fp16 matmuls into [128,1024] psum pair-tiles; causal mask is
          accumulated in psum by a tiny fp16 matmul (identity @ -30720*tril)
          so no post-exp masking is needed
  exp:    ACT -> fp16 et tiles [128, 2, 512]; off-diag pairs in one
          1024-wide instruction
  ctx:    fp16 matmuls into [65,512] psum; row 64 = softmax denominator
          (ones-column trick)
  norm:   DVE recip -> gpsimd partition_broadcast -> DVE mul -> cx2 fp16
  proj:   fp16 matmuls -> psum -> DVE copy -> fp16 out DMA
"""

import os

import numpy as np

B, S, D, H = 2, 2048, 1024, 16
HD = D // H  # 64
N_CORES = 8
NHC = 4  # heads per core
QB = 512  # query block
NQB = S // QB  # 4
NST = S // 128  # 16 s-tiles
NDC = D // 128  # 8 d-chunks

MASK_NEG = -30720.0
ET_BUFS = int(os.environ.get("ET_BUFS", "18"))
PSC_BUFS = int(os.environ.get("PSC_BUFS", "2"))
PCX_BUFS = int(os.environ.get("PCX_BUFS", "2"))
PM_BUFS = int(os.environ.get("PM_BUFS", "2"))
BDRAIN = os.environ.get("BDRAIN", "dve")  # dve | act
EDRAIN = os.environ.get("EDRAIN", "tailact")  # dve | act | tailact
BPULL = os.environ.get("BPULL", "0") == "1"
WARMUP = int(os.environ.get("WARMUP", "24"))

_CACHE = {}


def _build(with_bias=False):
    import concourse.bass as bass  # noqa: F401
    import concourse.tile as tile
    from concourse import bacc, mybir

    f32 = mybir.dt.float32
    fp16 = mybir.dt.float16
    EXP = mybir.ActivationFunctionType.Exp

    nc = bacc.Bacc("TRN2", target_bir_lowering=False, debug=False,
                   num_devices=N_CORES)

    x_d = nc.dram_tensor("x16", [128, NDC, S], fp16, kind="ExternalInput").ap()
    wqk_d = nc.dram_tensor("wqk16", [128, NDC, 512], fp16,
                           kind="ExternalInput").ap()
    wv_d = nc.dram_tensor("wv16", [128, NDC, 256], fp16,
                          kind="ExternalInput").ap()
    wp_d = nc.dram_tensor("wp16", [128, 2, D], fp16, kind="ExternalInput").ap()
    bqk_d = nc.dram_tensor("bqk", [128, 4], f32, kind="ExternalInput").ap()
    mneg_d = nc.dram_tensor("mneg", [128, 128], fp16,
                            kind="ExternalInput").ap()
    id_d = nc.dram_tensor("ident", [128, 128], fp16,
                          kind="ExternalInput").ap()
    out_d = nc.dram_tensor("out", [S, D], fp16, kind="ExternalOutput").ap()

    from contextlib import ExitStack
    with tile.TileContext(nc) as tc, ExitStack() as ctx:
        pool = lambda name, bufs: ctx.enter_context(
            tc.tile_pool(name=name, bufs=bufs))
        ppool = lambda name, bufs: ctx.enter_context(
            tc.tile_pool(name=name, bufs=bufs, space="PSUM"))

        stat = pool("stat", 1)
        expp = pool("expT", ET_BUFS)
        rtp = pool("rt", 3)
        bcp = pool("bcs", 3)
        outp = pool("outst", 3)
        psc = ppool("psc", PSC_BUFS)   # scores [128,1024] = 2 banks each
        pcx = ppool("pcx", PCX_BUFS)   # ctx [65,512]
        pm = ppool("pm", PM_BUFS)      # B/C/E matmuls [128,512]

        # ---- static sbuf tensors + loads (weights first, x in col blocks) ----
        mneg = stat.tile([128, 128], fp16, tag="mneg")
        nc.sync.dma_start(mneg[:], mneg_d[:])
        ident = stat.tile([128, 128], fp16, tag="ident")
        nc.sync.dma_start(ident[:], id_d[:])
        wqk16 = stat.tile([128, NDC, 512], fp16, tag="wqk16")
        nc.sync.dma_start(wqk16[:, :, 0:128], wqk_d[:, :, 0:128])
        x16 = stat.tile([128, NDC, S], fp16, tag="x16")
        nc.sync.dma_start(x16[:, :, 0:QB], x_d[:, :, 0:QB])
        nc.sync.dma_start(wqk16[:, :, 128:512], wqk_d[:, :, 128:512])
        wv16 = stat.tile([128, NDC, 256], fp16, tag="wv16")
        nc.sync.dma_start(wv16[:], wv_d[:])
        bqk = stat.tile([128, 4], f32, tag="bqk")
        nc.sync.dma_start(bqk[:], bqk_d[:])
        for nb in range(1, NQB):
            nc.sync.dma_start(x16[:, :, nb * QB:(nb + 1) * QB],
                              x_d[:, :, nb * QB:(nb + 1) * QB])
        wp16 = stat.tile([128, 2, D], fp16, tag="wp16")
        nc.sync.dma_start(wp16[:], wp_d[:])

        qkT = [stat.tile([128, S], fp16, tag=f"qkT{m}", name=f"qkT{m}")
               for m in range(4)]
        va2 = [stat.tile([128, 2, 4, 65], fp16, tag=f"va{t}", name=f"va{t}")
               for t in range(NST // 2)]
        cx2 = [stat.tile([128, 2, QB], fp16, tag=f"cx{qb}", name=f"cx{qb}")
               for qb in range(NQB)]
        for t in range(NST // 2):
            nc.gpsimd.memset(va2[t][:, :, :, 64:65], 1.0)

        def emit_B_group(qb, m):
            # qkT[m][:, qb_block] = (x @ wqk)^T m-block (+ bias)
            ps = pm.tile([128, QB], f32, tag="mm", name="psB")
            if qb == 0 and m == 0 and WARMUP:
                # spin the PE on the identity tile while x loads, so the
                # p-state ramp finishes before real work; stage B's
                # start=True overwrites the scratch result
                for w in range(WARMUP):
                    nc.tensor.matmul(ps[:, 0:128], lhsT=ident[:],
                                     rhs=ident[:], start=(w == 0),
                                     stop=(w == WARMUP - 1),
                                     skip_group_check=True)
            for d in range(NDC):
                nc.tensor.matmul(
                    ps[:],
                    lhsT=wqk16[:, d, m * 128:(m + 1) * 128],
                    rhs=x16[:, d, qb * QB:(qb + 1) * QB],
                    start=(d == 0), stop=(d == NDC - 1))
            dst = qkT[m][:, qb * QB:(qb + 1) * QB]
            if with_bias:
                nc.vector.tensor_scalar_add(dst, ps[:], bqk[:, m:m + 1])
            elif BDRAIN == "act":
                nc.scalar.copy(dst, ps[:])
            else:
                nc.vector.tensor_copy(dst, ps[:])

        def emit_B(qb):
            for m in range(4):
                emit_B_group(qb, m)

        def emit_C_group(qb, i):
            # va2 v-columns for s-tile 4qb+i
            st = 4 * qb + i
            ps = pm.tile([128, QB], f32, tag="mm", name="psC")
            for d in range(NDC):
                nc.tensor.matmul(
                    ps[:, 0:256],
                    lhsT=x16[:, d, st * 128:(st + 1) * 128],
                    rhs=wv16[:, d, :],
                    start=(d == 0), stop=(d == NDC - 1))
            nc.vector.tensor_copy(va2[st // 2][:, st % 2, :, 0:64],
                                  ps[:, 0:256])

        def emit_C(qb):
            for i in range(4):
                emit_C_group(qb, i)

        def emit_scores_exp(qb, h):
            qt = qkT[h // 2]
            kt = qkT[2 + h // 2]
            rb = 64 * (h % 2)
            q0 = qb * QB
            n_t = 2 * (qb + 1)
            ets = []
            for t in range(n_t):
                kc_e, kc_o = 2 * t, 2 * t + 1
                r_e = kc_e - 4 * qb
                diag = r_e >= 0
                c0_e = max(r_e, 0) * 128
                c0_o = max(r_e + 1, 0) * 128
                ps = psc.tile([128, 2 * QB], f32, tag="sc", name="sc")
                et = expp.tile([128, 2, QB], fp16, tag="et", name="et")
                for (kc, c0, off) in ((kc_e, c0_e, 0), (kc_o, c0_o, QB)):
                    nc.tensor.matmul(
                        ps[:, off + c0:off + QB],
                        lhsT=kt[rb:rb + 64, kc * 128:(kc + 1) * 128],
                        rhs=qt[rb:rb + 64, q0 + c0:q0 + QB],
                        start=True, stop=not diag, skip_group_check=True)
                    if diag:
                        nc.tensor.matmul(
                            ps[:, off + c0:off + c0 + 128],
                            lhsT=ident[:], rhs=mneg[:],
                            start=False, stop=True, skip_group_check=True)
                if not diag:
                    nc.scalar.activation(et[:, :, :], ps[:], EXP)
                else:
                    nc.scalar.activation(et[:, 0, c0_e:QB], ps[:, c0_e:QB],
                                         EXP)
                    nc.scalar.activation(et[:, 1, c0_o:QB],
                                         ps[:, QB + c0_o:2 * QB], EXP)
                ets.append((et, t, c0_e, c0_o, diag))
            return ets

        def emit_ctx_norm(qb, h, ets, pop_filler=None):
            pc = pcx.tile([65, QB], f32, tag="ctx", name="pc")
            last = len(ets) - 1
            for i, (et, t, c0_e, c0_o, diag) in enumerate(ets):
                va = va2[t]
                nc.tensor.matmul(
                    pc[:, c0_e:QB],
                    lhsT=va[:, 0, h, :], rhs=et[:, 0, c0_e:QB],
                    start=(i == 0), stop=False, skip_group_check=True)
                nc.tensor.matmul(
                    pc[:, c0_o:QB],
                    lhsT=va[:, 1, h, :], rhs=et[:, 1, c0_o:QB],
                    start=False, stop=(i == last), skip_group_check=True)
                if pop_filler is not None:
                    pop_filler()
            rt = rtp.tile([1, QB], fp16, tag="rt", name="rt")
            with nc.allow_low_precision(reason="fp16 softmax denominators"):
                nc.vector.reciprocal(rt[0:1, :], pc[64:65, :])
            bcs = bcp.tile([64, QB], fp16, tag="bcs", name="bcs")
            nc.gpsimd.partition_broadcast(bcs[:], rt[0:1, :])
            nc.vector.tensor_mul(
                cx2[qb][64 * (h % 2):64 * (h % 2) + 64, h // 2, :],
                pc[0:64, :], bcs[:])

        def emit_E(qb, i):
            # partial out projection for s-tile 4qb+i
            st = 4 * qb + i
            sc = i * 128
            so = outp.tile([128, D], fp16, tag="so", name="so")
            for nb2 in range(2):
                pp = pm.tile([128, QB], f32, tag="mm", name="psE")
                for j in range(2):
                    nc.tensor.matmul(
                        pp[:],
                        lhsT=cx2[qb][:, j, sc:sc + 128],
                        rhs=wp16[:, j, nb2 * QB:(nb2 + 1) * QB],
                        start=(j == 0), stop=(j == 1))
                if EDRAIN == "act" or (qb == NQB - 1 and EDRAIN == "tailact"):
                    nc.scalar.copy(so[:, nb2 * QB:(nb2 + 1) * QB], pp[:])
                else:
                    nc.vector.tensor_copy(so[:, nb2 * QB:(nb2 + 1) * QB],
                                          pp[:])
            nc.sync.dma_start(out_d[st * 128:(st + 1) * 128, :], so[:])

        pending = None
        pendE = None
        for qb in range(NQB):
            if not (BPULL and qb >= 2):
                emit_B(qb)
            emit_C(qb)
            for h in range(NHC):
                ets = emit_scores_exp(qb, h)
                if pending is not None:
                    emit_ctx_norm(*pending)
                pending = (qb, h, ets)
                if pendE is not None and h >= 2:
                    emit_E(pendE, 2 * (h - 2))
                    emit_E(pendE, 2 * (h - 2) + 1)
                if BPULL and qb >= 1 and qb < NQB - 1 and h >= 2:
                    emit_B_group(qb + 1, 2 * (h - 2))
                    emit_B_group(qb + 1, 2 * (h - 2) + 1)
            # E(qb) becomes eligible once all 4 heads' ctx written; emit its
            # 4 s-tiles spread across the next block's head loop
            pendE = qb
        emit_ctx_norm(*pending)
        for i in range(4):
            emit_E(NQB - 1, i)

    nc.compile()
    return nc


def _get_program(with_bias=False):
    key = with_bias
    if key not in _CACHE:
        _CACHE[key] = _build(with_bias)
    return _CACHE[key]


def make_in_maps(x, w_qkv, b_qkv, w_proj):
    """Build the 8 per-core input maps from full inputs."""
    fp16_np = np.dtype(np.float16)
    x = np.asarray(x, np.float32)
    w_qkv = np.asarray(w_qkv, np.float32)
    b_qkv = np.asarray(b_qkv, np.float32)
    w_proj = np.asarray(w_proj, np.float32)
    QS = 1.0 / np.sqrt(HD)  # fold softmax scale into wq
    mneg = (MASK_NEG * np.tril(np.ones((128, 128), np.float32), -1)
            ).astype(fp16_np)
    ident = np.eye(128, dtype=np.float32).astype(fp16_np)
    in_maps = []
    for c in range(N_CORES):
        b, hg = c // 4, c % 4
        hs = [hg * NHC + j for j in range(NHC)]
        wq = np.concatenate(
            [w_qkv[:, h * HD:(h + 1) * HD] for h in hs], 1) * QS
        wk = np.concatenate(
            [w_qkv[:, D + h * HD:D + (h + 1) * HD] for h in hs], 1)
        wqk = np.concatenate([wq, wk], 1)  # [1024, 512]
        bq = np.concatenate([b_qkv[h * HD:(h + 1) * HD] for h in hs]) * QS
        bk = np.concatenate(
            [b_qkv[D + h * HD:D + (h + 1) * HD] for h in hs])
        bqk = np.ascontiguousarray(
            np.concatenate([bq, bk]).reshape(4, 128).T).astype(np.float32)
        wv = w_qkv[:, 2 * D + hg * 256:2 * D + (hg + 1) * 256]
        wp = w_proj[hg * 256:(hg + 1) * 256, :]
        xt = np.ascontiguousarray(x[b].T)  # [1024, 2048]
        in_maps.append({
            "x16": np.ascontiguousarray(
                xt.reshape(NDC, 128, S).transpose(1, 0, 2)).astype(fp16_np),
            "wqk16": np.ascontiguousarray(
                wqk.reshape(NDC, 128, 512).transpose(1, 0, 2)).astype(fp16_np),
            "wv16": np.ascontiguousarray(
                wv.reshape(NDC, 128, 256).transpose(1, 0, 2)).astype(fp16_np),
            "wp16": np.ascontiguousarray(
                wp.reshape(2, 128, D).transpose(1, 0, 2)).astype(fp16_np),
            "bqk": bqk,
            "mneg": mneg,
            "ident": ident,
        })
    return in_maps


def assemble_output(results, b_qkv, b_proj, w_proj):
    """Sum per-core partials; add v-bias and proj-bias contributions."""
    out = np.zeros((B, S, D), np.float32)
    for c in range(N_CORES):
        out[c // 4] += results[c]["out"]
    bv = np.asarray(b_qkv, np.float32)[2 * D:]
    out += (bv @ np.asarray(w_proj, np.float32) +
            np.asarray(b_proj, np.float32))[None, None, :]
    return out


def kernel(x, w_qkv, b_qkv, w_proj, b_proj):
    from concourse.bass_utils import run_bass_kernel_spmd
    with_bias = bool(np.any(np.asarray(b_qkv, np.float32)[:2 * D]))
    nc = _get_program(with_bias)
    in_maps = make_in_maps(x, w_qkv, b_qkv, w_proj)
    res = run_bass_kernel_spmd(nc, in_maps, list(range(N_CORES)))
    return assemble_output(res.results, b_qkv, b_proj, w_proj)


# revision 33
# speedup vs baseline: 1.0521x; 1.0521x over previous
"""Causal self-attention (B=2, S=2048, D=1024, H=16) on 8 trn2 NeuronCores.

Sharding: core c -> batch b = c//4, head-group hg = c%4 (4 heads/core).
Each core computes the qkv projection for its heads, causal attention, and a
partial output projection (rows hg*256:(hg+1)*256 of w_proj). The host sums
the 4 partials per batch and adds the bias terms.

v2 design (fp16 datapath, fp32 psum):
  x16   [128, 8, S]   fp16  x[b]^T d-chunked      (DMA'd in 4 column blocks)
  wqk16 [128, 8, 512] fp16  (q cols pre-scaled by 1/sqrt(64))
  wv16  [128, 8, 256] fp16  wp16 [128, 2, 1024] fp16
  qkT[m] [128, S] fp16  = (x @ wqk)^T m-block (q pairs m=0,1; k pairs 2,3)
  va2[t] [128, 2, 4, 65] fp16  v for s-tile pair t, 65th col = ones
  scores: fp16 matmuls into [128,1024] psum pair-tiles; causal mask is
          accumulated in psum by a tiny fp16 matmul (identity @ -30720*tril)
          so no post-exp masking is needed
  exp:    ACT -> fp16 et tiles [128, 2, 512]; off-diag pairs in one
          1024-wide instruction
  ctx:    fp16 matmuls into [65,512] psum; row 64 = softmax denominator
          (ones-column trick)
  norm:   DVE recip -> gpsimd partition_broadcast -> DVE mul -> cx2 fp16
  proj:   fp16 matmuls -> psum -> DVE copy -> fp16 out DMA
"""

import os

import numpy as np

B, S, D, H = 2, 2048, 1024, 16
HD = D // H  # 64
N_CORES = 8
NHC = 4  # heads per core
QB = 512  # query block
NQB = S // QB  # 4
NST = S // 128  # 16 s-tiles
NDC = D // 128  # 8 d-chunks

MASK_NEG = -30720.0
ET_BUFS = int(os.environ.get("ET_BUFS", "18"))
PSC_BUFS = int(os.environ.get("PSC_BUFS", "2"))
PCX_BUFS = int(os.environ.get("PCX_BUFS", "2"))
PM_BUFS = int(os.environ.get("PM_BUFS", "2"))
BDRAIN = os.environ.get("BDRAIN", "dve")  # dve | act
EDRAIN = os.environ.get("EDRAIN", "tailact")  # dve | act | tailact
BPULL = os.environ.get("BPULL", "0") == "1"
WARMUP = int(os.environ.get("WARMUP", "36"))

_CACHE = {}


def _build(with_bias=False):
    import concourse.bass as bass  # noqa: F401
    import concourse.tile as tile
    from concourse import bacc, mybir

    f32 = mybir.dt.float32
    fp16 = mybir.dt.float16
    EXP = mybir.ActivationFunctionType.Exp

    nc = bacc.Bacc("TRN2", target_bir_lowering=False, debug=False,
                   num_devices=N_CORES)

    x_d = nc.dram_tensor("x16", [128, NDC, S], fp16, kind="ExternalInput").ap()
    wqk_d = nc.dram_tensor("wqk16", [128, NDC, 512], fp16,
                           kind="ExternalInput").ap()
    wv_d = nc.dram_tensor("wv16", [128, NDC, 256], fp16,
                          kind="ExternalInput").ap()
    wp_d = nc.dram_tensor("wp16", [128, 2, D], fp16, kind="ExternalInput").ap()
    bqk_d = nc.dram_tensor("bqk", [128, 4], f32, kind="ExternalInput").ap()
    mneg_d = nc.dram_tensor("mneg", [128, 128], fp16,
                            kind="ExternalInput").ap()
    id_d = nc.dram_tensor("ident", [128, 128], fp16,
                          kind="ExternalInput").ap()
    out_d = nc.dram_tensor("out", [S, D], fp16, kind="ExternalOutput").ap()

    from contextlib import ExitStack
    with tile.TileContext(nc) as tc, ExitStack() as ctx:
        pool = lambda name, bufs: ctx.enter_context(
            tc.tile_pool(name=name, bufs=bufs))
        ppool = lambda name, bufs: ctx.enter_context(
            tc.tile_pool(name=name, bufs=bufs, space="PSUM"))

        stat = pool("stat", 1)
        expp = pool("expT", ET_BUFS)
        rtp = pool("rt", 3)
        bcp = pool("bcs", 3)
        outp = pool("outst", 3)
        psc = ppool("psc", PSC_BUFS)   # scores [128,1024] = 2 banks each
        pcx = ppool("pcx", PCX_BUFS)   # ctx [65,512]
        pm = ppool("pm", PM_BUFS)      # B/C/E matmuls [128,512]

        # ---- static sbuf tensors + loads (weights first, x in col blocks) ----
        mneg = stat.tile([128, 128], fp16, tag="mneg")
        nc.sync.dma_start(mneg[:], mneg_d[:])
        ident = stat.tile([128, 128], fp16, tag="ident")
        nc.sync.dma_start(ident[:], id_d[:])
        wqk16 = stat.tile([128, NDC, 512], fp16, tag="wqk16")
        nc.sync.dma_start(wqk16[:, :, 0:128], wqk_d[:, :, 0:128])
        x16 = stat.tile([128, NDC, S], fp16, tag="x16")
        nc.sync.dma_start(x16[:, :, 0:QB], x_d[:, :, 0:QB])
        nc.sync.dma_start(wqk16[:, :, 128:512], wqk_d[:, :, 128:512])
        wv16 = stat.tile([128, NDC, 256], fp16, tag="wv16")
        nc.sync.dma_start(wv16[:], wv_d[:])
        bqk = stat.tile([128, 4], f32, tag="bqk")
        nc.sync.dma_start(bqk[:], bqk_d[:])
        for nb in range(1, NQB):
            nc.sync.dma_start(x16[:, :, nb * QB:(nb + 1) * QB],
                              x_d[:, :, nb * QB:(nb + 1) * QB])
        wp16 = stat.tile([128, 2, D], fp16, tag="wp16")
        nc.sync.dma_start(wp16[:], wp_d[:])

        qkT = [stat.tile([128, S], fp16, tag=f"qkT{m}", name=f"qkT{m}")
               for m in range(4)]
        va2 = [stat.tile([128, 2, 4, 65], fp16, tag=f"va{t}", name=f"va{t}")
               for t in range(NST // 2)]
        cx2 = [stat.tile([128, 2, QB], fp16, tag=f"cx{qb}", name=f"cx{qb}")
               for qb in range(NQB)]
        for t in range(NST // 2):
            nc.gpsimd.memset(va2[t][:, :, :, 64:65], 1.0)

        def emit_B_group(qb, m):
            # qkT[m][:, qb_block] = (x @ wqk)^T m-block (+ bias)
            ps = pm.tile([128, QB], f32, tag="mm", name="psB")
            if qb == 0 and m == 0 and WARMUP:
                # spin the PE on the identity tile while x loads, so the
                # p-state ramp finishes before real work; stage B's
                # start=True overwrites the scratch result
                for w in range(WARMUP):
                    nc.tensor.matmul(ps[:, 0:128], lhsT=ident[:],
                                     rhs=ident[:], start=(w == 0),
                                     stop=(w == WARMUP - 1),
                                     skip_group_check=True)
            for d in range(NDC):
                nc.tensor.matmul(
                    ps[:],
                    lhsT=wqk16[:, d, m * 128:(m + 1) * 128],
                    rhs=x16[:, d, qb * QB:(qb + 1) * QB],
                    start=(d == 0), stop=(d == NDC - 1))
            dst = qkT[m][:, qb * QB:(qb + 1) * QB]
            if with_bias:
                nc.vector.tensor_scalar_add(dst, ps[:], bqk[:, m:m + 1])
            elif BDRAIN == "act":
                nc.scalar.copy(dst, ps[:])
            else:
                nc.vector.tensor_copy(dst, ps[:])

        def emit_B(qb):
            for m in range(4):
                emit_B_group(qb, m)

        def emit_C_group(qb, i):
            # va2 v-columns for s-tile 4qb+i
            st = 4 * qb + i
            ps = pm.tile([128, QB], f32, tag="mm", name="psC")
            for d in range(NDC):
                nc.tensor.matmul(
                    ps[:, 0:256],
                    lhsT=x16[:, d, st * 128:(st + 1) * 128],
                    rhs=wv16[:, d, :],
                    start=(d == 0), stop=(d == NDC - 1))
            nc.vector.tensor_copy(va2[st // 2][:, st % 2, :, 0:64],
                                  ps[:, 0:256])

        def emit_C(qb):
            for i in range(4):
                emit_C_group(qb, i)

        def emit_scores_exp(qb, h):
            qt = qkT[h // 2]
            kt = qkT[2 + h // 2]
            rb = 64 * (h % 2)
            q0 = qb * QB
            n_t = 2 * (qb + 1)
            ets = []
            for t in range(n_t):
                kc_e, kc_o = 2 * t, 2 * t + 1
                r_e = kc_e - 4 * qb
                diag = r_e >= 0
                c0_e = max(r_e, 0) * 128
                c0_o = max(r_e + 1, 0) * 128
                ps = psc.tile([128, 2 * QB], f32, tag="sc", name="sc")
                et = expp.tile([128, 2, QB], fp16, tag="et", name="et")
                for (kc, c0, off) in ((kc_e, c0_e, 0), (kc_o, c0_o, QB)):
                    nc.tensor.matmul(
                        ps[:, off + c0:off + QB],
                        lhsT=kt[rb:rb + 64, kc * 128:(kc + 1) * 128],
                        rhs=qt[rb:rb + 64, q0 + c0:q0 + QB],
                        start=True, stop=not diag, skip_group_check=True)
                    if diag:
                        nc.tensor.matmul(
                            ps[:, off + c0:off + c0 + 128],
                            lhsT=ident[:], rhs=mneg[:],
                            start=False, stop=True, skip_group_check=True)
                if not diag:
                    nc.scalar.activation(et[:, :, :], ps[:], EXP)
                else:
                    nc.scalar.activation(et[:, 0, c0_e:QB], ps[:, c0_e:QB],
                                         EXP)
                    nc.scalar.activation(et[:, 1, c0_o:QB],
                                         ps[:, QB + c0_o:2 * QB], EXP)
                ets.append((et, t, c0_e, c0_o, diag))
            return ets

        def emit_ctx_norm(qb, h, ets, pop_filler=None):
            pc = pcx.tile([65, QB], f32, tag="ctx", name="pc")
            last = len(ets) - 1
            for i, (et, t, c0_e, c0_o, diag) in enumerate(ets):
                va = va2[t]
                nc.tensor.matmul(
                    pc[:, c0_e:QB],
                    lhsT=va[:, 0, h, :], rhs=et[:, 0, c0_e:QB],
                    start=(i == 0), stop=False, skip_group_check=True)
                nc.tensor.matmul(
                    pc[:, c0_o:QB],
                    lhsT=va[:, 1, h, :], rhs=et[:, 1, c0_o:QB],
                    start=False, stop=(i == last), skip_group_check=True)
                if pop_filler is not None:
                    pop_filler()
            rt = rtp.tile([1, QB], fp16, tag="rt", name="rt")
            with nc.allow_low_precision(reason="fp16 softmax denominators"):
                nc.vector.reciprocal(rt[0:1, :], pc[64:65, :])
            bcs = bcp.tile([64, QB], fp16, tag="bcs", name="bcs")
            nc.gpsimd.partition_broadcast(bcs[:], rt[0:1, :])
            nc.vector.tensor_mul(
                cx2[qb][64 * (h % 2):64 * (h % 2) + 64, h // 2, :],
                pc[0:64, :], bcs[:])

        def emit_E(qb, i):
            # partial out projection for s-tile 4qb+i
            st = 4 * qb + i
            sc = i * 128
            so = outp.tile([128, D], fp16, tag="so", name="so")
            for nb2 in range(2):
                pp = pm.tile([128, QB], f32, tag="mm", name="psE")
                for j in range(2):
                    nc.tensor.matmul(
                        pp[:],
                        lhsT=cx2[qb][:, j, sc:sc + 128],
                        rhs=wp16[:, j, nb2 * QB:(nb2 + 1) * QB],
                        start=(j == 0), stop=(j == 1))
                if EDRAIN == "act" or (qb == NQB - 1 and EDRAIN == "tailact"):
                    nc.scalar.copy(so[:, nb2 * QB:(nb2 + 1) * QB], pp[:])
                else:
                    nc.vector.tensor_copy(so[:, nb2 * QB:(nb2 + 1) * QB],
                                          pp[:])
            nc.sync.dma_start(out_d[st * 128:(st + 1) * 128, :], so[:])

        pending = None
        pendE = None
        for qb in range(NQB):
            if not (BPULL and qb >= 2):
                emit_B(qb)
            emit_C(qb)
            for h in range(NHC):
                ets = emit_scores_exp(qb, h)
                if pending is not None:
                    emit_ctx_norm(*pending)
                pending = (qb, h, ets)
                if pendE is not None and h >= 2:
                    emit_E(pendE, 2 * (h - 2))
                    emit_E(pendE, 2 * (h - 2) + 1)
                if BPULL and qb >= 1 and qb < NQB - 1 and h >= 2:
                    emit_B_group(qb + 1, 2 * (h - 2))
                    emit_B_group(qb + 1, 2 * (h - 2) + 1)
            # E(qb) becomes eligible once all 4 heads' ctx written; emit its
            # 4 s-tiles spread across the next block's head loop
            pendE = qb
        emit_ctx_norm(*pending)
        for i in range(4):
            emit_E(NQB - 1, i)

    nc.compile()
    return nc


def _get_program(with_bias=False):
    key = with_bias
    if key not in _CACHE:
        _CACHE[key] = _build(with_bias)
    return _CACHE[key]


def make_in_maps(x, w_qkv, b_qkv, w_proj):
    """Build the 8 per-core input maps from full inputs."""
    fp16_np = np.dtype(np.float16)
    x = np.asarray(x, np.float32)
    w_qkv = np.asarray(w_qkv, np.float32)
    b_qkv = np.asarray(b_qkv, np.float32)
    w_proj = np.asarray(w_proj, np.float32)
    QS = 1.0 / np.sqrt(HD)  # fold softmax scale into wq
    mneg = (MASK_NEG * np.tril(np.ones((128, 128), np.float32), -1)
            ).astype(fp16_np)
    ident = np.eye(128, dtype=np.float32).astype(fp16_np)
    in_maps = []
    for c in range(N_CORES):
        b, hg = c // 4, c % 4
        hs = [hg * NHC + j for j in range(NHC)]
        wq = np.concatenate(
            [w_qkv[:, h * HD:(h + 1) * HD] for h in hs], 1) * QS
        wk = np.concatenate(
            [w_qkv[:, D + h * HD:D + (h + 1) * HD] for h in hs], 1)
        wqk = np.concatenate([wq, wk], 1)  # [1024, 512]
        bq = np.concatenate([b_qkv[h * HD:(h + 1) * HD] for h in hs]) * QS
        bk = np.concatenate(
            [b_qkv[D + h * HD:D + (h + 1) * HD] for h in hs])
        bqk = np.ascontiguousarray(
            np.concatenate([bq, bk]).reshape(4, 128).T).astype(np.float32)
        wv = w_qkv[:, 2 * D + hg * 256:2 * D + (hg + 1) * 256]
        wp = w_proj[hg * 256:(hg + 1) * 256, :]
        xt = np.ascontiguousarray(x[b].T)  # [1024, 2048]
        in_maps.append({
            "x16": np.ascontiguousarray(
                xt.reshape(NDC, 128, S).transpose(1, 0, 2)).astype(fp16_np),
            "wqk16": np.ascontiguousarray(
                wqk.reshape(NDC, 128, 512).transpose(1, 0, 2)).astype(fp16_np),
            "wv16": np.ascontiguousarray(
                wv.reshape(NDC, 128, 256).transpose(1, 0, 2)).astype(fp16_np),
            "wp16": np.ascontiguousarray(
                wp.reshape(2, 128, D).transpose(1, 0, 2)).astype(fp16_np),
            "bqk": bqk,
            "mneg": mneg,
            "ident": ident,
        })
    return in_maps


def assemble_output(results, b_qkv, b_proj, w_proj):
    """Sum per-core partials; add v-bias and proj-bias contributions."""
    out = np.zeros((B, S, D), np.float32)
    for c in range(N_CORES):
        out[c // 4] += results[c]["out"]
    bv = np.asarray(b_qkv, np.float32)[2 * D:]
    out += (bv @ np.asarray(w_proj, np.float32) +
            np.asarray(b_proj, np.float32))[None, None, :]
    return out


def kernel(x, w_qkv, b_qkv, w_proj, b_proj):
    from concourse.bass_utils import run_bass_kernel_spmd
    with_bias = bool(np.any(np.asarray(b_qkv, np.float32)[:2 * D]))
    nc = _get_program(with_bias)
    in_maps = make_in_maps(x, w_qkv, b_qkv, w_proj)
    res = run_bass_kernel_spmd(nc, in_maps, list(range(N_CORES)))
    return assemble_output(res.results, b_qkv, b_proj, w_proj)
